# revision 8
# baseline (speedup 1.0000x reference)
"""ClusterNorm1d kernel for Trainium2 (Bass/Tile), 8-core data parallel.

out[b,d,k] = sum_e Std_inv[k,d,e] * (x[b,e,k] - mu[e,k])

Strategy:
  - Shard batch B=8192 across 8 cores (1024 rows each); replicate the small
    mu / Std_inv derived buffers on every core.
  - Per core, process batch tiles of 128 rows. Clusters are processed in
    PAIRS (k = j, j+64) so the contraction over e uses the full 128-row PE
    array: the pair's two 64x64 matrices are packed block-diagonally (in an
    interleaved row/col order c = 2e+p, n = 2d+p) into a 128x128 weight
    panel. The pair choice (j, j+64) makes the x slice for one pair a
    SINGLE strided free dim (offset j, stride 64, count 128), which the
    walrus matmul verifier requires for the stationary operand.
    Per pair:
       1. PE transpose of x slice [b=128, c=128] -> PSUM [c, b]
       2. DVE copy PSUM->SBUF fused with per-partition mu subtraction
       3. PE matmul: lhsT = (x-mu)^T [c, b], rhs = W_j [c, n] -> PSUM [b, n]
       4. ACT copy PSUM->SBUF output staging at stride-64 offsets so the
          final DMA out is fully contiguous.
"""

import numpy as np

B, D, K = 8192, 64, 128
N_CORES = 8
B_SHARD = B // N_CORES  # 1024
P = 128                 # SBUF partitions = batch tile size
NPAIR = K // 2          # 64 cluster pairs: (j, j+64)

_cache = {}


def _build_nc(b_shard):
    import concourse.tile as tile
    from concourse import bacc, mybir
    from concourse.masks import make_identity

    f32 = mybir.dt.float32
    nc = bacc.Bacc("TRN2", target_bir_lowering=False)

    x_d = nc.dram_tensor("x", [b_shard, D * K], f32, kind="ExternalInput")
    w_d = nc.dram_tensor("w", [2 * D, NPAIR, 2 * D], f32, kind="ExternalInput")
    mu_d = nc.dram_tensor("mu", [2 * D, NPAIR], f32, kind="ExternalInput")
    o_d = nc.dram_tensor("out", [b_shard, D * K], f32, kind="ExternalOutput")

    ntiles = b_shard // P

    with tile.TileContext(nc) as tc:
        with (
            tc.tile_pool(name="consts", bufs=1) as consts,
            tc.tile_pool(name="xin", bufs=2) as xin,
            tc.tile_pool(name="xt", bufs=4) as xtp,
            tc.tile_pool(name="oout", bufs=2) as oout,
            tc.tile_pool(name="psT", bufs=4, space="PSUM") as psT,
            tc.tile_pool(name="psO", bufs=4, space="PSUM") as psO,
        ):
            ident = consts.tile([P, P], f32)
            make_identity(nc, ident)
            w_sb = consts.tile([2 * D, NPAIR, 2 * D], f32)
            nc.sync.dma_start(out=w_sb, in_=w_d[:])
            mu_sb = consts.tile([2 * D, NPAIR], f32)
            nc.sync.dma_start(out=mu_sb, in_=mu_d[:])

            # Walrus allows only ONE semaphore wait per ISA instruction.
            # Warm-up ops make each engine observe the const-load semaphores
            # (Pool for the identity, DMA lanes for w/mu) once, so steady
            # state instructions carry at most one wait each.
            warm_ps = psT.tile([P, P], f32, tag="xt_ps")
            nc.tensor.transpose(warm_ps, ident, ident)          # PE <- Pool
            warm_mm = psO.tile([P, P], f32, tag="o_ps")
            nc.tensor.matmul(warm_mm, lhsT=ident, rhs=w_sb[:, 0, :])  # PE <- w
            scratch = consts.tile([2 * D, 1], f32)
            nc.vector.tensor_copy(scratch, mu_sb[:, 0:1])       # DVE <- mu

            for t in range(ntiles):
                x_t = xin.tile([P, D * K], f32, tag="x_t")
                nc.sync.dma_start(out=x_t, in_=x_d[t * P:(t + 1) * P])
                # view so that [:, j, :] = offset j, stride 64, count 128
                x_w = x_t.rearrange("b (t s) -> b s t", s=NPAIR)
                o_t = oout.tile([P, D * K], f32)
                o_w = o_t.rearrange("b (t s) -> b s t", s=NPAIR)
                # Sacrificial write: absorbs the "previous out-DMA finished"
                # buffer-release wait so the first real ACT copy below only
                # waits on PE.
                nc.scalar.copy(out=o_t[:, 0:1], in_=ident[:, 0:1])
                for j in range(NPAIR):
                    xt_ps = psT.tile([P, P], f32)
                    nc.tensor.transpose(xt_ps, x_w[:, j, :], ident)
                    xt_s = xtp.tile([P, P], f32)
                    nc.vector.tensor_scalar_sub(xt_s, xt_ps, mu_sb[:, j:j + 1])
                    o_ps = psO.tile([P, P], f32)
                    nc.tensor.matmul(o_ps, lhsT=xt_s, rhs=w_sb[:, j, :])
                    nc.scalar.copy(out=o_w[:, j, :], in_=o_ps)
                nc.sync.dma_start(out=o_d[t * P:(t + 1) * P], in_=o_t)

    nc.compile()
    return nc


def _host_prep(mu_track, Std_inv_track):
    """Pack W [2D, NPAIR, 2D] with c=2e+p, n=2d+p, pair j = (k=j, k=j+64),
    and mu [2D, NPAIR]."""
    W = np.zeros((2 * D, NPAIR, 2 * D), dtype=np.float32)
    W6 = W.reshape(D, 2, NPAIR, D, 2)                 # [e, p, j, d, p']
    S_r = np.ascontiguousarray(Std_inv_track, dtype=np.float32).reshape(
        2, NPAIR, D, D)                               # [p, j, d, e]
    W6[:, 0, :, :, 0] = S_r[0].transpose(2, 0, 1)     # [e, j, d]
    W6[:, 1, :, :, 1] = S_r[1].transpose(2, 0, 1)
    mu_sb = np.ascontiguousarray(mu_track, dtype=np.float32).reshape(
        D, 2, NPAIR).reshape(2 * D, NPAIR)            # [2e+p, j]
    return W, mu_sb


def kernel(x, mu_track, Std_inv_track):
    from concourse.bass_utils import run_bass_kernel_spmd

    x = np.ascontiguousarray(x, dtype=np.float32).reshape(B, D * K)
    W, mu_sb = _host_prep(mu_track, Std_inv_track)

    if "nc" not in _cache:
        _cache["nc"] = _build_nc(B_SHARD)
    nc = _cache["nc"]

    in_maps = []
    for i in range(N_CORES):
        in_maps.append({
            "x": x[i * B_SHARD:(i + 1) * B_SHARD],
            "w": W,
            "mu": mu_sb,
        })
    res = run_bass_kernel_spmd(nc, in_maps, core_ids=list(range(N_CORES)))
    out = np.concatenate([r["out"] for r in res.results], axis=0)
    return out.reshape(B, D, K)
